# revision 34
# baseline (speedup 1.0000x reference)
"""Trainium2 Bass kernel for multi-head attention (b=4, n=2048, dim=256, H=8, D=32).

Sharding: 8 cores = 4 batches x 2 query-halves. Each core computes the full
attention for its 1024 query rows against all 2048 keys of its batch.
No collectives; host slices inputs and concatenates outputs.

Design (ACT-exp-bound: the softmax exp on ScalarE is ~1.08us per [128,1024]
tile x 128 tiles; PE work per unit is kept below that even at the cold
1.2 GHz clock gate):
  c1s/c2b --PE transpose--> c1T/c2T (f32r); qT = Wq^T c1T, kT = Wk^T c2T
  (evacuated to bf16 so the S^T weight loads get FWL); v = c2T^T Wv (bf16,
  with a fused ones column).
  loop qh (q-512 halves) -> pr (head pairs) -> kb (16 k-blocks of 128):
    S^T [128k, 2x512q]: two K=32 bf16 matmuls ROW-tiled at strips b0/b1
      (concurrent on the PE sub-arrays)
    P^T = exp(0.125*S^T) on ACT (PSUM -> bf16 SBUF)
    AV: two M=33 matmuls COL-tiled: even head -> PSUM partitions 0:33, odd
      head -> 64:97 (concurrent, separate XBUS streams); the ones column
      accumulates the softmax denominator on partitions 32 / 96
  per (qh, pr) block: evacuate av [97,512] -> SBUF, DMA-bounce the two raw
    denominator rows to DRAM and broadcast them to partition bands 0:32 /
    64:96 of a bc tile, ONE reciprocal over [96,512] (DVE cost is free-size
    only; dead band 32:64 is primed to 1.0), multiply -> out_sb.
  y = out @ Wo + bo: one K=96 matmul per head pair per q-block (dead rows
    32:64 are zero in out_sb and Wo), all at tile position (0,0) --
    accumulation groups with non-zero tile_position hang the HW. qb 0-3 are
    interleaved into the qh=1 phase; qb 4-7 run in the tail.
  The prologue (transposes + projections) is deadline-scheduled into early
  units and uses the y PSUM pool so it never breaks the S^T double-buffer;
  DMA issue order / queues are arranged so the first c1/c2 chunks and
  weights land before the pipeline needs them.
"""

import os
import sys

for p in ("/opt/trn_rl_repo", "/opt/pypackages"):
    if p not in sys.path:
        sys.path.insert(0, p)

from contextlib import ExitStack

import numpy as np

import concourse.bass as bass
import concourse.bacc as bacc
import concourse.mybir as mybir
import concourse.tile as tile
from concourse.masks import make_identity

P = 128
NQ = 1024          # per-core query rows
NK = 2048          # keys
DIM = 256
H = 8
D = 32
SCALE = 64 ** -0.5  # 0.125, matches reference
FP32 = mybir.dt.float32
F32R = mybir.dt.float32r
BF16 = mybir.dt.bfloat16

N_CORES = 8
NKB = NK // P      # 16 k-blocks
QW = 512           # q columns per unit


def _mm(ap):
    """Bitcast an fp32 AP to float32r for full-rate PE matmuls."""
    return ap.bitcast(F32R)


def _strip_pe_self_waits(nc):
    """Drop PE-sem waits from PE matmuls. The PE is strictly in-order with a
    single PSUM write port and never reads PSUM nor writes SBUF, so a PE
    instruction can never race another PE instruction; Tile still emits these
    same-engine waits, and matmul instructions only support one sync wait."""
    pe = mybir.EngineType.PE
    for f in nc.m.functions:
        for bb in f.blocks:
            for inst in bb.instructions:
                if type(inst).__name__ != "InstMatmult" or inst.engine != pe:
                    continue
                si = inst.sync_info
                if si is None:
                    continue
                ws = [w for w in si.on_wait if not str(w.ant_name).startswith("PE_")]
                if len(ws) != len(si.on_wait):
                    si.on_wait = ws
                    inst.sync_info = si


def _strip_redundant_waits(nc):
    """ACT is also strictly in-order: drop Activation-sem self-waits from
    ACTIVATE instructions (WAW on cycled SBUF output slots is FIFO-safe).
    Output stores: drop DMAHW lane-bookkeeping waits (they only order the
    store against an unrelated earlier input DMA that reused the same
    round-robin completion lane; the data dependency is the DVE wait)."""
    act = mybir.EngineType.Activation
    store_names = set(getattr(nc, "_y_store_names", ()))
    for f in nc.m.functions:
        for bb in f.blocks:
            for inst in bb.instructions:
                si = getattr(inst, "sync_info", None)
                if si is None or len(si.on_wait) <= 1:
                    continue
                tn = type(inst).__name__
                if tn == "InstActivation" and inst.engine == act:
                    ws = [w for w in si.on_wait
                          if not str(w.ant_name).startswith("Activation")]
                elif tn == "InstDMACopy" and inst.name in store_names:
                    ws = [w for w in si.on_wait
                          if not str(w.ant_name).startswith("DMAHW")]
                else:
                    continue
                if len(ws) != len(si.on_wait):
                    si.on_wait = ws
                    inst.sync_info = si


def build_nc(mm_cast=_mm):
    nc = bacc.Bacc()
    c1s = nc.dram_tensor("c1s", [NQ, DIM], F32R, kind="ExternalInput")
    c2b = nc.dram_tensor("c2b", [NK, DIM], F32R, kind="ExternalInput")
    wq = nc.dram_tensor("wq", [DIM, DIM], FP32, kind="ExternalInput")
    wk = nc.dram_tensor("wk", [DIM, DIM], FP32, kind="ExternalInput")
    wv = nc.dram_tensor("wv", [DIM, DIM], FP32, kind="ExternalInput")
    wo = nc.dram_tensor("wo", [DIM, DIM], FP32, kind="ExternalInput")
    bo = nc.dram_tensor("bo", [DIM], FP32, kind="ExternalInput")
    y = nc.dram_tensor("y", [NQ, DIM], FP32, kind="ExternalOutput")
    rdd = nc.dram_tensor("rdd", [8, 2, QW], FP32)

    with tile.TileContext(nc) as tc, ExitStack() as ctx:
        _body(tc, ctx, c1s, c2b, wq, wk, wv, wo, bo, y, rdd, mm_cast)
    if os.environ.get("KERNEL_STRIP_WAITS", "1") == "1":
        _strip_pe_self_waits(nc)
        _strip_redundant_waits(nc)
    nc.finalize()
    return nc


def _body(tc, ctx, c1s, c2b, wq, wk, wv, wo, bo, y, rdd, mm):
    nc = tc.nc
    Exp = mybir.ActivationFunctionType.Exp
    MULT = mybir.AluOpType.mult
    ADD = mybir.AluOpType.add

    persist = ctx.enter_context(tc.tile_pool(name="persist", bufs=1))
    stage = ctx.enter_context(tc.tile_pool(name="stage", bufs=1))

    # ---- constants (the identity gates the transposes: keep it first on
    # the gpsimd queue, ahead of any big memset) ----
    ident_gp = persist.tile([P, P], FP32, tag="ident_gp")
    make_identity(nc, ident_gp)
    ident = persist.tile([P, P], F32R, tag="ident")
    nc.vector.tensor_copy(out=ident, in_=ident_gp)
    v4 = persist.tile([P, NKB, H, D + 1], BF16, tag="v4")
    # only the ones-column needs the fill; v copies overwrite cols 0..D-1
    nc.gpsimd.memset(v4[:, :, :, D:D + 1], 1.0)
    out_q = [persist.tile([96, 4, QW], BF16, tag=f"outq{i}", name=f"outq{i}")
             for i in range(2)]

    # DMA order = completion order: small weight tensors first (they gate the
    # q/k projections), then the first c1/c2 chunks, then the rest.
    wq_sb = persist.tile([P, 2, DIM], F32R, tag="wq")
    wk_sb = persist.tile([P, 2, DIM], F32R, tag="wk")
    wv_sb = persist.tile([P, 2, DIM], F32R, tag="wv")
    # Two HWDGE queues run in parallel but each serializes its own DMAs:
    # big activation chunks go on the SP queue in consumption order, the
    # small weight tensors stream concurrently on the ACT queue.
    c1nat = stage.tile([P, NQ // P, DIM], F32R, tag="cnat")
    c1r = c1s.rearrange("(n p) d -> p n d", p=P)
    c2nat = stage.tile([P, NK // P, DIM], F32R, tag="c2nat")
    c2r = c2b.rearrange("(n p) d -> p n d", p=P)
    nc.sync.dma_start(out=c1nat[:, 0:4, :], in_=c1r[:, 0:4, :])
    w_stages = []
    for wi, (w_dram, w_sb) in enumerate(((wq, wq_sb), (wk, wk_sb), (wv, wv_sb))):
        wst = stage.tile([P, 2, DIM], FP32, tag=f"wst{wi}", name=f"wst{wi}")
        nc.scalar.dma_start(out=wst,
                            in_=w_dram.rearrange("(c p) f -> p c f", p=P))
        w_stages.append((w_sb, wst))
    nc.sync.dma_start(out=c2nat[:, 0:4, :], in_=c2r[:, 0:4, :])
    nc.sync.dma_start(out=c2nat[:, 4:8, :], in_=c2r[:, 4:8, :])
    nc.sync.dma_start(out=c1nat[:, 4:8, :], in_=c1r[:, 4:8, :])
    for ch in range(2, 4):
        nc.sync.dma_start(out=c2nat[:, 4 * ch:4 * ch + 4, :],
                          in_=c2r[:, 4 * ch:4 * ch + 4, :])
    # Wo: even heads' [d=32, f] blocks at partitions 0:32, odd heads at 64:96.
    # Rows 32:64 are zeroed: the y matmul contracts K=96 in one shot (dead
    # rows contribute nothing), keeping every accumulation at tile (0, 0) --
    # accumulation groups with non-zero row tile_position hang the HW.
    wo_sb = persist.tile([96, 4, DIM], BF16, tag="wo")
    nc.gpsimd.memset(wo_sb[D:64], 0.0)
    nc.gpsimd.memset(out_q[0][D:64], 0.0)  # dead rows of the K=96 y matmul
    nc.gpsimd.memset(out_q[1][D:64], 0.0)
    wo_r = wo.rearrange("(g e d) f -> e d g f", e=2, d=D)
    wost = stage.tile([96, 4, DIM], FP32, tag="wost")
    nc.scalar.dma_start(out=wost[0:D, :, :], in_=wo_r[0])
    nc.scalar.dma_start(out=wost[64:64 + D, :, :], in_=wo_r[1])
    # bias broadcast to all partitions
    bo_st = stage.tile([P, DIM], FP32, tag="bo_st")
    nc.gpsimd.dma_start(out=bo_st, in_=bo[:].partition_broadcast(P))
    bo_bc = persist.tile([P, DIM], FP32, tag="bo")
    # warm the ACT exp table while the prologue runs (input is the identity,
    # which is ready immediately -- the implicit table load has no waits)
    exp_warm = persist.tile([1, 4], FP32, tag="exp_warm")
    nc.scalar.activation(out=exp_warm, in_=ident_gp[0:1, 0:4],
                         func=Exp, scale=float(SCALE))

    pt_pool = ctx.enter_context(tc.tile_pool(name="pt", bufs=6))
    avs_pool = ctx.enter_context(tc.tile_pool(name="avs", bufs=2))
    yout = ctx.enter_context(tc.tile_pool(name="yout", bufs=8))
    bc_tiles = [persist.tile([96, QW], FP32, tag=f"bc{i}", name=f"bc{i}")
                for i in range(2)]
    for bt in bc_tiles:
        nc.gpsimd.memset(bt[D:64], 1.0)

    # ---- persistent activations ----
    c1T = [persist.tile([P, NQ], F32R, tag=f"c1T{i}", name=f"c1T{i}") for i in range(2)]
    c2T = [persist.tile([P, NK], F32R, tag=f"c2T{i}", name=f"c2T{i}") for i in range(2)]
    qT = [persist.tile([P, NQ], BF16, tag=f"qT{i}", name=f"qT{i}") for i in range(2)]
    kT = [persist.tile([P, NK], BF16, tag=f"kT{i}", name=f"kT{i}") for i in range(2)]

    with tc.tile_pool(name="st_psum", bufs=2, space="PSUM") as st_psum, \
         tc.tile_pool(name="av_psum", bufs=2, space="PSUM") as av_psum, \
         tc.tile_pool(name="y_psum", bufs=2, space="PSUM") as y_psum:

        def c1tp(n4, fh):
            """Transpose c1 rows [n4*512, n4*512+512) for feature half fh."""
            tp = y_psum.tile([P, 512], F32R, tag="y", name="tp")
            for j in range(4):
                nc.tensor.transpose(tp[:, j * P:(j + 1) * P],
                                    c1nat[:, 4 * n4 + j, fh * P:(fh + 1) * P],
                                    ident)
            nc.vector.tensor_copy(
                out=c1T[fh][:, n4 * 512:(n4 + 1) * 512], in_=tp[:, :512])

        def c2tp(n4, fh):
            tp = y_psum.tile([P, 512], F32R, tag="y", name="tp")
            for j in range(4):
                nc.tensor.transpose(tp[:, j * P:(j + 1) * P],
                                    c2nat[:, 4 * n4 + j, fh * P:(fh + 1) * P],
                                    ident)
            nc.vector.tensor_copy(
                out=c2T[fh][:, n4 * 512:(n4 + 1) * 512], in_=tp[:, :512])

        def qt_proj(fb, qb):
            pp = y_psum.tile([P, 512], FP32, tag="y", name="pp")
            for c in range(2):
                nc.tensor.matmul(
                    pp, lhsT=wq_sb[:, c, fb * P:(fb + 1) * P],
                    rhs=c1T[c][:, qb * 512:(qb + 1) * 512],
                    start=(c == 0), stop=(c == 1),
                )
            nc.vector.tensor_copy(out=qT[fb][:, qb * 512:(qb + 1) * 512], in_=pp)

        def kt_proj(fb, nb):
            pp = y_psum.tile([P, 512], FP32, tag="y", name="pp")
            for c in range(2):
                nc.tensor.matmul(
                    pp, lhsT=wk_sb[:, c, fb * P:(fb + 1) * P],
                    rhs=c2T[c][:, nb * 512:(nb + 1) * 512],
                    start=(c == 0), stop=(c == 1),
                )
            nc.vector.tensor_copy(out=kT[fb][:, nb * 512:(nb + 1) * 512], in_=pp)

        def v_proj(kb):
            pp = y_psum.tile([P, 512], FP32, tag="y", name="pp")
            for c in range(2):
                nc.tensor.matmul(
                    pp[:, :DIM], lhsT=c2T[c][:, kb * P:(kb + 1) * P],
                    rhs=wv_sb[:, c, :], start=(c == 0), stop=(c == 1),
                )
            nc.vector.tensor_copy(
                out=v4[:, kb, :, 0:D],
                in_=pp[:, :DIM].rearrange("p (h d) -> p h d", d=D),
            )

        def y_proj(qb):
            """y[qb*128:(qb+1)*128] = out @ Wo + bo. One K=96 matmul per head
            pair (dead rows 32:64 are zero in both operands)."""
            yp = y_psum.tile([P, 512], FP32, tag="y")
            for pr in range(4):
                nc.tensor.matmul(
                    yp[:, :DIM],
                    lhsT=out_q[qb // 4][:, pr, (qb % 4) * P:(qb % 4 + 1) * P],
                    rhs=wo_sb[:, pr, :],
                    start=(pr == 0), stop=(pr == 3),
                    skip_group_check=True,
                )
            ys = yout.tile([P, DIM], FP32, tag="ys")
            nc.vector.tensor_tensor(out=ys, in0=yp[:, :DIM], in1=bo_bc, op=ADD)
            st_inst = nc.sync.dma_start(out=y[qb * P:(qb + 1) * P, :], in_=ys)
            nc._y_store_names = getattr(nc, "_y_store_names", []) + [st_inst.ins.name]


        # pre-unit-0 prologue: unit 0's needs plus what fits in the DMA
        # shadow. Weight staging copies are placed just-in-time so the DVE
        # FIFO never blocks the transpose evacuations behind a late DMA.
        c1tp(0, 0)
        c1tp(0, 1)
        nc.vector.tensor_copy(out=wq_sb, in_=w_stages[0][1])
        qt_proj(0, 0)
        c2tp(0, 0)
        c2tp(0, 1)
        nc.vector.tensor_copy(out=wk_sb, in_=w_stages[1][1])
        kt_proj(0, 0)
        nc.vector.tensor_copy(out=wv_sb, in_=w_stages[2][1])
        v_proj(0)
        nc.vector.tensor_copy(out=wo_sb[0:D], in_=wost[0:D])
        nc.vector.tensor_copy(out=wo_sb[64:64 + D], in_=wost[64:64 + D])
        nc.vector.tensor_copy(out=bo_bc, in_=bo_st)

        # deadline-scheduled leftover prologue work, injected into early units
        extras = {}

        def sched(u, fn, *a):
            extras.setdefault(u, []).append((fn, a))

        sched(0, v_proj, 1)       # AV(kb) is emitted at unit kb+2
        sched(0, c2tp, 1, 0)
        sched(1, c2tp, 1, 1)
        sched(1, v_proj, 2)
        sched(2, kt_proj, 0, 1)   # S^T needs kT(0,1) at u4
        sched(2, v_proj, 3)
        sched(3, v_proj, 4)
        sched(4, c2tp, 2, 0)
        sched(4, v_proj, 5)
        sched(5, c2tp, 2, 1)
        sched(6, kt_proj, 0, 2)   # needed at u8
        sched(6, v_proj, 6)
        sched(7, v_proj, 7)
        sched(8, c2tp, 3, 0)
        sched(9, c2tp, 3, 1)
        sched(10, kt_proj, 0, 3)  # needed at u12
        sched(10, v_proj, 8)
        sched(11, v_proj, 9)
        sched(12, v_proj, 10)
        sched(13, v_proj, 11)
        for kb in range(12, NKB):
            sched(kb, v_proj, kb)       # 2-unit margin before AV at kb+2
        sched(17, c1tp, 1, 0)
        sched(18, c1tp, 1, 1)
        sched(19, qt_proj, 1, 0)        # needed at u32
        for nb in range(4):
            sched(21 + 2 * nb, kt_proj, 1, nb)  # kt(1,nb) needed at u32+4nb
        sched(40, qt_proj, 0, 1)        # needed at u64
        sched(46, qt_proj, 1, 1)        # needed at u96
        # y projections for the first q-half, injected once qh=0 outputs
        # land. The negative high_priority offset stops the Tile scheduler
        # from hoisting them to the instant their drain deps resolve (which
        # bubbles the PE behind the drain chain).
        def y_late(qb):
            with tc.high_priority(offset=-150):
                y_proj(qb)
        for i, qb in enumerate(range(4)):
            sched(88 + 6 * i, y_late, qb)

        def drain_block(qh, pr, av):
            """Evacuate + normalize one (qh, pr) attention block. The raw
            denominators are broadcast (GpSimd, SBUF->SBUF) and a single
            reciprocal covers both 32-row bands (DVE cost is free-size only;
            dead rows 32:64 are primed to 1.0 and stay 1.0)."""
            blk = qh * 4 + pr
            avs = avs_pool.tile([97, QW], F32R, tag="avs", name="avs")
            nc.vector.tensor_copy(out=avs[0:D + 1], in_=av[0:D + 1])
            nc.vector.tensor_copy(out=avs[64:97], in_=av[64:97])
            bcb = bc_tiles[blk % 2]
            # SP HWDGE (idle mid-kernel) has lower issue+completion latency
            # than the GpSimd SWDGE path for these four small hops
            nc.sync.dma_start(out=rdd[blk, 0], in_=avs[D:D + 1].bitcast(FP32))
            nc.sync.dma_start(out=rdd[blk, 1], in_=avs[96:97].bitcast(FP32))
            nc.sync.dma_start(out=bcb[0:D],
                              in_=rdd[blk, 0].partition_broadcast(D))
            nc.sync.dma_start(out=bcb[64:64 + D],
                              in_=rdd[blk, 1].partition_broadcast(D))
            with nc.allow_low_precision(reason="denominator reciprocal"):
                nc.vector.reciprocal(out=bcb[0:96], in_=bcb[0:96])
            nc.vector.tensor_tensor(out=out_q[qh][0:D, pr, :],
                                    in0=avs[0:D], in1=bcb[0:D], op=MULT)
            nc.vector.tensor_tensor(out=out_q[qh][64:64 + D, pr, :],
                                    in0=avs[64:64 + D], in1=bcb[64:64 + D],
                                    op=MULT)

        # ---- attention: qh-major; row-tiled S^T, col-tiled AV ----
        pending = []

        def emit_av(ent):
            pt, qh, pr, kb, av = ent
            h0 = 2 * pr
            nc.tensor.matmul(
                av[0:D + 1, :],
                lhsT=v4[:, kb, h0, :],
                rhs=pt[:, 0:QW],
                start=(kb == 0), stop=(kb == NKB - 1),
                skip_group_check=True,
            )
            nc.tensor.matmul(
                av[64:64 + D + 1, :],
                lhsT=v4[:, kb, h0 + 1, :],
                rhs=pt[:, QW:2 * QW],
                start=(kb == 0), stop=(kb == NKB - 1),
                skip_group_check=True,
            )
            if kb == NKB - 1:
                drain_block(qh, pr, av)

        u = 0
        for qh in range(2):
            for pr in range(4):
                h0 = 2 * pr
                ht = h0 // 4
                b0 = (h0 % 4) * 32
                b1 = b0 + 32
                av = av_psum.tile([97, QW], FP32, tag="av")
                qs = slice(qh * QW, (qh + 1) * QW)
                for kb in range(NKB):
                    for fn, a in extras.get(u, []):
                        fn(*a)
                    st = st_psum.tile([P, 1024], FP32, tag="st")
                    nc.tensor.matmul(
                        st[:, 0:QW],
                        lhsT=kT[ht][b0:b0 + 32, kb * P:(kb + 1) * P],
                        rhs=qT[ht][b0:b0 + 32, qs],
                        start=True, stop=True, tile_position=(b0, 0),
                    )
                    nc.tensor.matmul(
                        st[:, QW:2 * QW],
                        lhsT=kT[ht][b1:b1 + 32, kb * P:(kb + 1) * P],
                        rhs=qT[ht][b1:b1 + 32, qs],
                        start=True, stop=True, tile_position=(b1, 0),
                    )
                    pt = pt_pool.tile([P, 1024], BF16, tag="pt")
                    nc.scalar.activation(out=pt, in_=st, func=Exp,
                                         scale=float(SCALE))
                    pending.append((pt, qh, pr, kb, av))
                    if len(pending) > 2:
                        emit_av(pending.pop(0))
                    u += 1
        for ent in pending:
            emit_av(ent)
        pending = []

        # ---- second-half output projection (first half ran as extras) ----
        for qb in range(4, 8):
            y_proj(qb)


_NC_CACHE = None


def _get_nc():
    global _NC_CACHE
    if _NC_CACHE is None:
        _NC_CACHE = build_nc()
    return _NC_CACHE


def make_in_maps(c2, c1, Wq, Wk, Wv, Wo, bo):
    c1 = np.asarray(c1, np.float32)
    c2 = np.asarray(c2, np.float32)
    Wq, Wk, Wv, Wo, bo = (np.asarray(a, np.float32) for a in (Wq, Wk, Wv, Wo, bo))
    in_maps = []
    for core in range(N_CORES):
        b, qh = core // 2, core % 2
        in_maps.append({
            "c1s": np.ascontiguousarray(c1[b, qh * NQ:(qh + 1) * NQ, :]),
            "c2b": np.ascontiguousarray(c2[b]),
            "wq": Wq, "wk": Wk, "wv": Wv, "wo": Wo, "bo": bo,
        })
    return in_maps


def assemble(results):
    out = np.empty((4, 2 * NQ, DIM), np.float32)
    for core in range(N_CORES):
        b, qh = core // 2, core % 2
        out[b, qh * NQ:(qh + 1) * NQ, :] = results[core]["y"]
    return out


def run_spmd(inputs, trace=False, **kwargs):
    from concourse.bass_utils import run_bass_kernel_spmd

    nc = _get_nc()
    in_maps = make_in_maps(**inputs)
    res = run_bass_kernel_spmd(
        nc, in_maps, core_ids=list(range(N_CORES)), trace=trace, **kwargs
    )
    return assemble(res.results), res


def kernel(c2, c1, Wq, Wk, Wv, Wo, bo):
    out, _ = run_spmd(dict(c2=c2, c1=c1, Wq=Wq, Wk=Wk, Wv=Wv, Wo=Wo, bo=bo))
    return out


# revision 35
# speedup vs baseline: 1.1898x; 1.1898x over previous
"""Trainium2 Bass kernel for multi-head attention (b=4, n=2048, dim=256, H=8, D=32).

Sharding: 8 cores = 4 batches x 2 query-halves. Each core computes the full
attention for its 1024 query rows against all 2048 keys of its batch.
No collectives; host slices inputs and concatenates outputs.

Design (ACT-exp-bound: the softmax exp on ScalarE is ~1.08us per [128,1024]
tile x 128 tiles; PE work per unit is kept below that even at the cold
1.2 GHz clock gate):
  c1s/c2b --PE transpose--> c1T/c2T (f32r); qT = Wq^T c1T, kT = Wk^T c2T
  (evacuated to bf16 so the S^T weight loads get FWL); v = c2T^T Wv (bf16,
  with a fused ones column).
  loop qh (q-512 halves) -> pr (head pairs) -> kb (16 k-blocks of 128):
    S^T [128k, 2x512q]: two K=32 bf16 matmuls ROW-tiled at strips b0/b1
      (concurrent on the PE sub-arrays)
    P^T = exp(0.125*S^T) on ACT (PSUM -> bf16 SBUF)
    AV: two M=33 matmuls COL-tiled: even head -> PSUM partitions 0:33, odd
      head -> 64:97 (concurrent, separate XBUS streams); the ones column
      accumulates the softmax denominator on partitions 32 / 96
  per (qh, pr) block: evacuate av [97,512] -> SBUF, DMA-bounce the two raw
    denominator rows to DRAM and broadcast them to partition bands 0:32 /
    64:96 of a bc tile, ONE reciprocal over [96,512] (DVE cost is free-size
    only; dead band 32:64 is primed to 1.0), multiply -> out_sb.
  y = out @ Wo + bo: one K=96 matmul per head pair per q-block (dead rows
    32:64 are zero in out_sb and Wo), all at tile position (0,0) --
    accumulation groups with non-zero tile_position hang the HW. qb 0-3 are
    interleaved into the qh=1 phase; qb 4-7 run in the tail.
  The prologue (transposes + projections) is deadline-scheduled into early
  units and uses the y PSUM pool so it never breaks the S^T double-buffer;
  DMA issue order / queues are arranged so the first c1/c2 chunks and
  weights land before the pipeline needs them.
"""

import os
import sys

for p in ("/opt/trn_rl_repo", "/opt/pypackages"):
    if p not in sys.path:
        sys.path.insert(0, p)

from contextlib import ExitStack

import numpy as np

import concourse.bass as bass
import concourse.bacc as bacc
import concourse.mybir as mybir
import concourse.tile as tile
from concourse.masks import make_identity

P = 128
NQ = 1024          # per-core query rows
NK = 2048          # keys
DIM = 256
H = 8
D = 32
SCALE = 64 ** -0.5  # 0.125, matches reference
FP32 = mybir.dt.float32
F32R = mybir.dt.float32r
BF16 = mybir.dt.bfloat16

N_CORES = 8
NKB = NK // P      # 16 k-blocks
QW = 512           # q columns per unit


def _mm(ap):
    """Bitcast an fp32 AP to float32r for full-rate PE matmuls."""
    return ap.bitcast(F32R)


def _strip_pe_self_waits(nc):
    """Drop PE-sem waits from PE matmuls. The PE is strictly in-order with a
    single PSUM write port and never reads PSUM nor writes SBUF, so a PE
    instruction can never race another PE instruction; Tile still emits these
    same-engine waits, and matmul instructions only support one sync wait."""
    pe = mybir.EngineType.PE
    for f in nc.m.functions:
        for bb in f.blocks:
            for inst in bb.instructions:
                if type(inst).__name__ != "InstMatmult" or inst.engine != pe:
                    continue
                si = inst.sync_info
                if si is None:
                    continue
                ws = [w for w in si.on_wait if not str(w.ant_name).startswith("PE_")]
                if len(ws) != len(si.on_wait):
                    si.on_wait = ws
                    inst.sync_info = si


def _strip_redundant_waits(nc):
    """ACT is also strictly in-order: drop Activation-sem self-waits from
    ACTIVATE instructions (WAW on cycled SBUF output slots is FIFO-safe).
    Output stores: drop DMAHW lane-bookkeeping waits (they only order the
    store against an unrelated earlier input DMA that reused the same
    round-robin completion lane; the data dependency is the DVE wait)."""
    act = mybir.EngineType.Activation
    store_names = set(getattr(nc, "_y_store_names", ()))
    for f in nc.m.functions:
        for bb in f.blocks:
            for inst in bb.instructions:
                si = getattr(inst, "sync_info", None)
                if si is None or len(si.on_wait) <= 1:
                    continue
                tn = type(inst).__name__
                if tn == "InstActivation" and inst.engine == act:
                    ws = [w for w in si.on_wait
                          if not str(w.ant_name).startswith("Activation")]
                elif tn == "InstDMACopy" and inst.name in store_names:
                    ws = [w for w in si.on_wait
                          if not str(w.ant_name).startswith("DMAHW")]
                else:
                    continue
                if len(ws) != len(si.on_wait):
                    si.on_wait = ws
                    inst.sync_info = si


def build_nc(mm_cast=_mm):
    nc = bacc.Bacc()
    c1s = nc.dram_tensor("c1s", [NQ, DIM], F32R, kind="ExternalInput")
    c2b = nc.dram_tensor("c2b", [NK, DIM], F32R, kind="ExternalInput")
    wq = nc.dram_tensor("wq", [DIM, DIM], FP32, kind="ExternalInput")
    wk = nc.dram_tensor("wk", [DIM, DIM], FP32, kind="ExternalInput")
    wv = nc.dram_tensor("wv", [DIM, DIM], FP32, kind="ExternalInput")
    wo = nc.dram_tensor("wo", [DIM, DIM], FP32, kind="ExternalInput")
    bo = nc.dram_tensor("bo", [DIM], FP32, kind="ExternalInput")
    y = nc.dram_tensor("y", [NQ, DIM], FP32, kind="ExternalOutput")
    rdd = nc.dram_tensor("rdd", [8, 2, QW], FP32)

    with tile.TileContext(nc) as tc, ExitStack() as ctx:
        _body(tc, ctx, c1s, c2b, wq, wk, wv, wo, bo, y, rdd, mm_cast)
    if os.environ.get("KERNEL_STRIP_WAITS", "1") == "1":
        _strip_pe_self_waits(nc)
        _strip_redundant_waits(nc)
    nc.finalize()
    return nc


def _body(tc, ctx, c1s, c2b, wq, wk, wv, wo, bo, y, rdd, mm):
    nc = tc.nc
    Exp = mybir.ActivationFunctionType.Exp
    MULT = mybir.AluOpType.mult
    ADD = mybir.AluOpType.add

    persist = ctx.enter_context(tc.tile_pool(name="persist", bufs=1))
    stage = ctx.enter_context(tc.tile_pool(name="stage", bufs=1))

    # ---- constants (the identity gates the transposes: keep it first on
    # the gpsimd queue, ahead of any big memset) ----
    ident_gp = persist.tile([P, P], FP32, tag="ident_gp")
    make_identity(nc, ident_gp)
    ident = persist.tile([P, P], F32R, tag="ident")
    nc.vector.tensor_copy(out=ident, in_=ident_gp)
    v4 = persist.tile([P, NKB, H, D + 1], BF16, tag="v4")
    # only the ones-column needs the fill; v copies overwrite cols 0..D-1
    nc.gpsimd.memset(v4[:, :, :, D:D + 1], 1.0)
    out_q = [persist.tile([96, 4, QW], BF16, tag=f"outq{i}", name=f"outq{i}")
             for i in range(2)]

    # DMA order = completion order: small weight tensors first (they gate the
    # q/k projections), then the first c1/c2 chunks, then the rest.
    wq_sb = persist.tile([P, 2, DIM], F32R, tag="wq")
    wk_sb = persist.tile([P, 2, DIM], F32R, tag="wk")
    wv_sb = persist.tile([P, 2, DIM], F32R, tag="wv")
    # Two HWDGE queues run in parallel but each serializes its own DMAs:
    # big activation chunks go on the SP queue in consumption order, the
    # small weight tensors stream concurrently on the ACT queue.
    c1nat = stage.tile([P, NQ // P, DIM], F32R, tag="cnat")
    c1r = c1s.rearrange("(n p) d -> p n d", p=P)
    c2nat = stage.tile([P, NK // P, DIM], F32R, tag="c2nat")
    c2r = c2b.rearrange("(n p) d -> p n d", p=P)
    nc.sync.dma_start(out=c1nat[:, 0:4, :], in_=c1r[:, 0:4, :])
    w_stages = []
    for wi, (w_dram, w_sb) in enumerate(((wq, wq_sb), (wk, wk_sb), (wv, wv_sb))):
        wst = stage.tile([P, 2, DIM], FP32, tag=f"wst{wi}", name=f"wst{wi}")
        nc.scalar.dma_start(out=wst,
                            in_=w_dram.rearrange("(c p) f -> p c f", p=P))
        w_stages.append((w_sb, wst))
    nc.sync.dma_start(out=c2nat[:, 0:4, :], in_=c2r[:, 0:4, :])
    nc.sync.dma_start(out=c2nat[:, 4:8, :], in_=c2r[:, 4:8, :])
    nc.sync.dma_start(out=c1nat[:, 4:8, :], in_=c1r[:, 4:8, :])
    for ch in range(2, 4):
        nc.sync.dma_start(out=c2nat[:, 4 * ch:4 * ch + 4, :],
                          in_=c2r[:, 4 * ch:4 * ch + 4, :])
    # Wo: even heads' [d=32, f] blocks at partitions 0:32, odd heads at 64:96.
    # Rows 32:64 are zeroed: the y matmul contracts K=96 in one shot (dead
    # rows contribute nothing), keeping every accumulation at tile (0, 0) --
    # accumulation groups with non-zero row tile_position hang the HW.
    wo_sb = persist.tile([96, 4, DIM], BF16, tag="wo")
    nc.gpsimd.memset(wo_sb[D:64], 0.0)
    nc.gpsimd.memset(out_q[0][D:64], 0.0)  # dead rows of the K=96 y matmul
    nc.gpsimd.memset(out_q[1][D:64], 0.0)
    wo_r = wo.rearrange("(g e d) f -> e d g f", e=2, d=D)
    wost = stage.tile([96, 4, DIM], FP32, tag="wost")
    nc.scalar.dma_start(out=wost[0:D, :, :], in_=wo_r[0])
    nc.scalar.dma_start(out=wost[64:64 + D, :, :], in_=wo_r[1])
    # bias broadcast to all partitions
    bo_st = stage.tile([P, DIM], FP32, tag="bo_st")
    nc.gpsimd.dma_start(out=bo_st, in_=bo[:].partition_broadcast(P))
    bo_bc = persist.tile([P, DIM], FP32, tag="bo")
    # warm the ACT exp table while the prologue runs (input is the identity,
    # which is ready immediately -- the implicit table load has no waits)
    exp_warm = persist.tile([1, 4], FP32, tag="exp_warm")
    nc.scalar.activation(out=exp_warm, in_=ident_gp[0:1, 0:4],
                         func=Exp, scale=float(SCALE))

    pt_pool = ctx.enter_context(tc.tile_pool(name="pt", bufs=6))
    avs_pool = ctx.enter_context(tc.tile_pool(name="avs", bufs=2))
    yout = ctx.enter_context(tc.tile_pool(name="yout", bufs=8))
    bc_tiles = [persist.tile([96, QW], FP32, tag=f"bc{i}", name=f"bc{i}")
                for i in range(4)]
    for bt in bc_tiles:
        nc.gpsimd.memset(bt[D:64], 1.0)

    # ---- persistent activations ----
    c1T = [persist.tile([P, NQ], F32R, tag=f"c1T{i}", name=f"c1T{i}") for i in range(2)]
    c2T = [persist.tile([P, NK], F32R, tag=f"c2T{i}", name=f"c2T{i}") for i in range(2)]
    qT = [persist.tile([P, NQ], BF16, tag=f"qT{i}", name=f"qT{i}") for i in range(2)]
    kT = [persist.tile([P, NK], BF16, tag=f"kT{i}", name=f"kT{i}") for i in range(2)]

    with tc.tile_pool(name="st_psum", bufs=2, space="PSUM") as st_psum, \
         tc.tile_pool(name="av_psum", bufs=2, space="PSUM") as av_psum, \
         tc.tile_pool(name="y_psum", bufs=2, space="PSUM") as y_psum:

        def c1tp(n4, fh):
            """Transpose c1 rows [n4*512, n4*512+512) for feature half fh."""
            tp = y_psum.tile([P, 512], F32R, tag="y", name="tp")
            for j in range(4):
                nc.tensor.transpose(tp[:, j * P:(j + 1) * P],
                                    c1nat[:, 4 * n4 + j, fh * P:(fh + 1) * P],
                                    ident)
            nc.vector.tensor_copy(
                out=c1T[fh][:, n4 * 512:(n4 + 1) * 512], in_=tp[:, :512])

        def c2tp(n4, fh):
            tp = y_psum.tile([P, 512], F32R, tag="y", name="tp")
            for j in range(4):
                nc.tensor.transpose(tp[:, j * P:(j + 1) * P],
                                    c2nat[:, 4 * n4 + j, fh * P:(fh + 1) * P],
                                    ident)
            nc.vector.tensor_copy(
                out=c2T[fh][:, n4 * 512:(n4 + 1) * 512], in_=tp[:, :512])

        def qt_proj(fb, qb):
            pp = y_psum.tile([P, 512], FP32, tag="y", name="pp")
            for c in range(2):
                nc.tensor.matmul(
                    pp, lhsT=wq_sb[:, c, fb * P:(fb + 1) * P],
                    rhs=c1T[c][:, qb * 512:(qb + 1) * 512],
                    start=(c == 0), stop=(c == 1),
                )
            nc.vector.tensor_copy(out=qT[fb][:, qb * 512:(qb + 1) * 512], in_=pp)

        def kt_proj(fb, nb):
            pp = y_psum.tile([P, 512], FP32, tag="y", name="pp")
            for c in range(2):
                nc.tensor.matmul(
                    pp, lhsT=wk_sb[:, c, fb * P:(fb + 1) * P],
                    rhs=c2T[c][:, nb * 512:(nb + 1) * 512],
                    start=(c == 0), stop=(c == 1),
                )
            nc.vector.tensor_copy(out=kT[fb][:, nb * 512:(nb + 1) * 512], in_=pp)

        def v_proj(kb):
            pp = y_psum.tile([P, 512], FP32, tag="y", name="pp")
            for c in range(2):
                nc.tensor.matmul(
                    pp[:, :DIM], lhsT=c2T[c][:, kb * P:(kb + 1) * P],
                    rhs=wv_sb[:, c, :], start=(c == 0), stop=(c == 1),
                )
            nc.vector.tensor_copy(
                out=v4[:, kb, :, 0:D],
                in_=pp[:, :DIM].rearrange("p (h d) -> p h d", d=D),
            )

        def y_proj(qb):
            """y[qb*128:(qb+1)*128] = out @ Wo + bo. One K=96 matmul per head
            pair (dead rows 32:64 are zero in both operands)."""
            yp = y_psum.tile([P, 512], FP32, tag="y")
            for pr in range(4):
                nc.tensor.matmul(
                    yp[:, :DIM],
                    lhsT=out_q[qb // 4][:, pr, (qb % 4) * P:(qb % 4 + 1) * P],
                    rhs=wo_sb[:, pr, :],
                    start=(pr == 0), stop=(pr == 3),
                    skip_group_check=True,
                )
            ys = yout.tile([P, DIM], FP32, tag="ys")
            nc.vector.tensor_tensor(out=ys, in0=yp[:, :DIM], in1=bo_bc, op=ADD)
            st_inst = nc.sync.dma_start(out=y[qb * P:(qb + 1) * P, :], in_=ys)
            nc._y_store_names = getattr(nc, "_y_store_names", []) + [st_inst.ins.name]


        # pre-unit-0 prologue: unit 0's needs plus what fits in the DMA
        # shadow. Weight staging copies are placed just-in-time so the DVE
        # FIFO never blocks the transpose evacuations behind a late DMA.
        c1tp(0, 0)
        c1tp(0, 1)
        nc.vector.tensor_copy(out=wq_sb, in_=w_stages[0][1])
        qt_proj(0, 0)
        c2tp(0, 0)
        c2tp(0, 1)
        nc.vector.tensor_copy(out=wk_sb, in_=w_stages[1][1])
        kt_proj(0, 0)
        nc.vector.tensor_copy(out=wv_sb, in_=w_stages[2][1])
        v_proj(0)
        nc.vector.tensor_copy(out=wo_sb[0:D], in_=wost[0:D])
        nc.vector.tensor_copy(out=wo_sb[64:64 + D], in_=wost[64:64 + D])
        nc.vector.tensor_copy(out=bo_bc, in_=bo_st)

        # deadline-scheduled leftover prologue work, injected into early units
        extras = {}

        def sched(u, fn, *a):
            extras.setdefault(u, []).append((fn, a))

        sched(0, v_proj, 1)       # AV(kb) is emitted at unit kb+2
        sched(0, c2tp, 1, 0)
        sched(1, c2tp, 1, 1)
        sched(1, v_proj, 2)
        sched(2, kt_proj, 0, 1)   # S^T needs kT(0,1) at u4
        sched(2, v_proj, 3)
        sched(3, v_proj, 4)
        sched(4, c2tp, 2, 0)
        sched(4, v_proj, 5)
        sched(5, c2tp, 2, 1)
        sched(6, kt_proj, 0, 2)   # needed at u8
        sched(6, v_proj, 6)
        sched(7, v_proj, 7)
        sched(8, c2tp, 3, 0)
        sched(9, c2tp, 3, 1)
        sched(10, kt_proj, 0, 3)  # needed at u12
        sched(10, v_proj, 8)
        sched(11, v_proj, 9)
        sched(12, v_proj, 10)
        sched(13, v_proj, 11)
        for kb in range(12, NKB):
            sched(kb, v_proj, kb)       # 2-unit margin before AV at kb+2
        sched(17, c1tp, 1, 0)
        sched(18, c1tp, 1, 1)
        sched(19, qt_proj, 1, 0)        # needed at u32
        for nb in range(4):
            sched(21 + 2 * nb, kt_proj, 1, nb)  # kt(1,nb) needed at u32+4nb
        sched(40, qt_proj, 0, 1)        # needed at u64
        sched(46, qt_proj, 1, 1)        # needed at u96
        # y projections for the first q-half, injected once qh=0 outputs
        # land. The negative high_priority offset stops the Tile scheduler
        # from hoisting them to the instant their drain deps resolve (which
        # bubbles the PE behind the drain chain).
        def y_late(qb):
            with tc.high_priority(offset=-150):
                y_proj(qb)
        for i, qb in enumerate(range(4)):
            sched(88 + 6 * i, y_late, qb)

        def drain_block(qh, pr, av):
            """Evacuate + normalize one (qh, pr) attention block. The raw
            denominators are broadcast (GpSimd, SBUF->SBUF) and a single
            reciprocal covers both 32-row bands (DVE cost is free-size only;
            dead rows 32:64 are primed to 1.0 and stay 1.0)."""
            blk = qh * 4 + pr
            avs = avs_pool.tile([97, QW], F32R, tag="avs", name="avs")
            nc.vector.tensor_copy(out=avs[0:D + 1], in_=av[0:D + 1])
            nc.vector.tensor_copy(out=avs[64:97], in_=av[64:97])
            bcb = bc_tiles[blk % 4]
            # SP HWDGE (idle mid-kernel) has lower issue+completion latency
            # than the GpSimd SWDGE path for these four small hops
            nc.sync.dma_start(out=rdd[blk, 0], in_=avs[D:D + 1].bitcast(FP32))
            nc.sync.dma_start(out=rdd[blk, 1], in_=avs[96:97].bitcast(FP32))
            nc.sync.dma_start(out=bcb[0:D],
                              in_=rdd[blk, 0].partition_broadcast(D))
            nc.sync.dma_start(out=bcb[64:64 + D],
                              in_=rdd[blk, 1].partition_broadcast(D))
            with nc.allow_low_precision(reason="denominator reciprocal"):
                nc.vector.reciprocal(out=bcb[0:96], in_=bcb[0:96])
            nc.vector.tensor_tensor(out=out_q[qh][0:D, pr, :],
                                    in0=avs[0:D], in1=bcb[0:D], op=MULT)
            nc.vector.tensor_tensor(out=out_q[qh][64:64 + D, pr, :],
                                    in0=avs[64:64 + D], in1=bcb[64:64 + D],
                                    op=MULT)

        # ---- attention: qh-major; row-tiled S^T, col-tiled AV ----
        pending = []

        def emit_av(ent):
            pt, qh, pr, kb, av = ent
            h0 = 2 * pr
            nc.tensor.matmul(
                av[0:D + 1, :],
                lhsT=v4[:, kb, h0, :],
                rhs=pt[:, 0:QW],
                start=(kb == 0), stop=(kb == NKB - 1),
                skip_group_check=True,
            )
            nc.tensor.matmul(
                av[64:64 + D + 1, :],
                lhsT=v4[:, kb, h0 + 1, :],
                rhs=pt[:, QW:2 * QW],
                start=(kb == 0), stop=(kb == NKB - 1),
                skip_group_check=True,
            )
            if kb == NKB - 1:
                drain_block(qh, pr, av)

        u = 0
        for qh in range(2):
            for pr in range(4):
                h0 = 2 * pr
                ht = h0 // 4
                b0 = (h0 % 4) * 32
                b1 = b0 + 32
                av = av_psum.tile([97, QW], FP32, tag="av")
                qs = slice(qh * QW, (qh + 1) * QW)
                for kb in range(NKB):
                    for fn, a in extras.get(u, []):
                        fn(*a)
                    st = st_psum.tile([P, 1024], FP32, tag="st")
                    nc.tensor.matmul(
                        st[:, 0:QW],
                        lhsT=kT[ht][b0:b0 + 32, kb * P:(kb + 1) * P],
                        rhs=qT[ht][b0:b0 + 32, qs],
                        start=True, stop=True, tile_position=(b0, 0),
                    )
                    nc.tensor.matmul(
                        st[:, QW:2 * QW],
                        lhsT=kT[ht][b1:b1 + 32, kb * P:(kb + 1) * P],
                        rhs=qT[ht][b1:b1 + 32, qs],
                        start=True, stop=True, tile_position=(b1, 0),
                    )
                    pt = pt_pool.tile([P, 1024], BF16, tag="pt")
                    nc.scalar.activation(out=pt, in_=st, func=Exp,
                                         scale=float(SCALE))
                    pending.append((pt, qh, pr, kb, av))
                    if len(pending) > 2:
                        emit_av(pending.pop(0))
                    u += 1
        for ent in pending:
            emit_av(ent)
        pending = []

        # ---- second-half output projection (first half ran as extras) ----
        for qb in range(4, 8):
            y_proj(qb)


_NC_CACHE = None


def _get_nc():
    global _NC_CACHE
    if _NC_CACHE is None:
        _NC_CACHE = build_nc()
    return _NC_CACHE


def make_in_maps(c2, c1, Wq, Wk, Wv, Wo, bo):
    c1 = np.asarray(c1, np.float32)
    c2 = np.asarray(c2, np.float32)
    Wq, Wk, Wv, Wo, bo = (np.asarray(a, np.float32) for a in (Wq, Wk, Wv, Wo, bo))
    in_maps = []
    for core in range(N_CORES):
        b, qh = core // 2, core % 2
        in_maps.append({
            "c1s": np.ascontiguousarray(c1[b, qh * NQ:(qh + 1) * NQ, :]),
            "c2b": np.ascontiguousarray(c2[b]),
            "wq": Wq, "wk": Wk, "wv": Wv, "wo": Wo, "bo": bo,
        })
    return in_maps


def assemble(results):
    out = np.empty((4, 2 * NQ, DIM), np.float32)
    for core in range(N_CORES):
        b, qh = core // 2, core % 2
        out[b, qh * NQ:(qh + 1) * NQ, :] = results[core]["y"]
    return out


def run_spmd(inputs, trace=False, **kwargs):
    from concourse.bass_utils import run_bass_kernel_spmd

    nc = _get_nc()
    in_maps = make_in_maps(**inputs)
    res = run_bass_kernel_spmd(
        nc, in_maps, core_ids=list(range(N_CORES)), trace=trace, **kwargs
    )
    return assemble(res.results), res


def kernel(c2, c1, Wq, Wk, Wv, Wo, bo):
    out, _ = run_spmd(dict(c2=c2, c1=c1, Wq=Wq, Wk=Wk, Wv=Wv, Wo=Wo, bo=bo))
    return out
